# revision 14
# baseline (speedup 1.0000x reference)
"""Trainium2 Bass kernel: LayerNorm + multi-head self-attention + residual.

Computes, per batch b:
    xn = LayerNorm(x[b]) * g + b
    q/k/v = xn @ W{q,k,v}.T + b{q,k,v}      (16 heads, dh=64)
    attn  = softmax(q k^T + maskbias, over keys)
    out   = x + (attn @ (v*mask)) reshaped

Sharding over 8 cores: batch (2-way) x head-group (4-way, 4 heads each).
Each core gets full x[b] (for LayerNorm) plus its 256-column slice of the
Q/K/V weights, and produces a [2048, 256] slice of the output.

On-device dataflow (per core):
  1. LN in [n, d] layout (bn_stats/bn_aggr), affine -> fp16, PE-transpose
     128x128 blocks into xnT [d, n] (g/b applied per-partition post-transpose).
  2. Projections: Q^T,K^T as [c, n] (c = head dims stacked in pairs of heads),
     V as [m, c] with bias via a rank-1 ones x bias matmul; V stored with a
     ones column per head (65 cols) to produce softmax denominators for free.
  3. Attention per head-pair: S^T[m-chunk, n-slice] = K^T.T @ Q^T (K=64
     contraction, the two heads auto-pack into row groups 0-63/64-127);
     P = exp(S^T + maskbias[m]) on ScalarE (bias is per-partition; softmax
     needs no max-subtraction since |S| <~ 50 fits fp32/bf16 range);
     Y^T[65, n] += V'[m,65].T @ P accumulated over m-chunks (row 64 = denom).
  4. PE-transpose Y^T 128-blocks -> [n, 65]; multiply by 1/denom (now a
     per-partition scalar), add residual x, DMA out.
"""

import sys

for _p in ("/opt/trn_rl_repo",):
    if _p not in sys.path:
        sys.path.insert(0, _p)

import numpy as np

import concourse.bacc as bacc
import concourse.bass as bass
import concourse.mybir as mybir
import concourse.tile as tile
from concourse.masks import make_identity

F32 = mybir.dt.float32
F16 = mybir.dt.float16
BF16 = mybir.dt.bfloat16

T = 2048          # sequence length
D = 1024          # model dim
HC = 4            # heads per core
DH = 64           # head dim
CC = HC * DH      # columns per core (256)
NC = T // 128     # 16 n/m chunks of 128
NJ = T // 512     # 4 n-slices of 512
DC = D // 128     # 8 d chunks

_CACHE = {}


def build_bass():
    # Bacc (not plain Bass): its finalize() runs generate_event_semaphores,
    # which splits multi-waits into EventSemaphore instructions — walrus
    # rejects >1 sync wait on most engine instruction structs.
    nc = bacc.Bacc()

    x_d = nc.declare_dram_parameter("x", [T, D], F32, isOutput=False)
    xres_d = nc.declare_dram_parameter("xres", [T, CC], F32, isOutput=False)
    wqt_d = nc.declare_dram_parameter("wqt", [D, CC], F16, isOutput=False)
    wkt_d = nc.declare_dram_parameter("wkt", [D, CC], F16, isOutput=False)
    wvt_d = nc.declare_dram_parameter("wvt", [D, CC], F16, isOutput=False)
    lngb_d = nc.declare_dram_parameter("lngb", [128, DC, 2], F32,
                                       isOutput=False)
    bq_d = nc.declare_dram_parameter("bq2", [128, 2], F32, isOutput=False)
    bk_d = nc.declare_dram_parameter("bk2", [128, 2], F32, isOutput=False)
    bvr_d = nc.declare_dram_parameter("bvr", [1, CC], F16, isOutput=False)
    mb_d = nc.declare_dram_parameter("mbias", [128, NC], F32, isOutput=False)
    mm_d = nc.declare_dram_parameter("mmul", [128, NC], F32, isOutput=False)
    out_d = nc.declare_dram_parameter("out", [T, CC], F32, isOutput=True)

    with tile.TileContext(nc) as tc:
        _body(tc, x_d, xres_d, wqt_d, wkt_d, wvt_d, lngb_d,
              bq_d, bk_d, bvr_d, mb_d, mm_d, out_d)
    nc.finalize()
    return nc


def _body(tc, x_d, xres_d, wqt_d, wkt_d, wvt_d, lngb_d,
          bq_d, bk_d, bvr_d, mb_d, mm_d, out_d):
    nc = tc.nc
    import contextlib
    ctx = contextlib.ExitStack()
    with ctx:
        consts = ctx.enter_context(tc.tile_pool(name="consts", bufs=1))
        persist = ctx.enter_context(tc.tile_pool(name="persist", bufs=1))
        xcpool = ctx.enter_context(tc.tile_pool(name="xcpool", bufs=2))
        stats = ctx.enter_context(tc.tile_pool(name="stats", bufs=4))
        ppool = ctx.enter_context(tc.tile_pool(name="ppool", bufs=4))
        ytpool = ctx.enter_context(tc.tile_pool(name="ytpool", bufs=3))
        recpool = ctx.enter_context(tc.tile_pool(name="recpool", bufs=4))
        outpool = ctx.enter_context(tc.tile_pool(name="outpool", bufs=3))
        mmpsum = ctx.enter_context(tc.tile_pool(name="mmpsum", bufs=4, space="PSUM"))
        tppsum = ctx.enter_context(tc.tile_pool(name="tppsum", bufs=2, space="PSUM"))
        avpsum = ctx.enter_context(tc.tile_pool(name="avpsum", bufs=2, space="PSUM"))

        # ---- constants -------------------------------------------------
        wq_sb = consts.tile([128, DC, CC], F16)
        wk_sb = consts.tile([128, DC, CC], F16)
        wv_sb = consts.tile([128, DC, CC], F16)
        nc.sync.dma_start(wq_sb, wqt_d[:].rearrange("(o p) c -> p o c", p=128))
        nc.sync.dma_start(wk_sb, wkt_d[:].rearrange("(o p) c -> p o c", p=128))
        nc.sync.dma_start(wv_sb, wvt_d[:].rearrange("(o p) c -> p o c", p=128))
        lngb_t = consts.tile([128, DC, 2], F32)
        nc.sync.dma_start(lngb_t, lngb_d[:])
        bq_t = consts.tile([128, 2], F32)
        bk_t = consts.tile([128, 2], F32)
        nc.sync.dma_start(bq_t, bq_d[:])
        nc.sync.dma_start(bk_t, bk_d[:])
        bvr_t = consts.tile([1, CC], F16)
        nc.sync.dma_start(bvr_t, bvr_d[:])
        mb_t = consts.tile([128, NC], F32)
        mm_t = consts.tile([128, NC], F32)
        nc.sync.dma_start(mb_t, mb_d[:])
        nc.sync.dma_start(mm_t, mm_d[:])

        # "touch" const tensors on the engines that later read them via
        # scalar-pointer operands: TensorScalarPtr/Activation structs can
        # encode only one sync wait, so the DMA-completion wait must be
        # absorbed here (later ops inherit it via engine program order).
        touch_v = consts.tile([128, 1], F32)
        nc.vector.tensor_copy(touch_v, lngb_t[:, 0, 0:1])
        nc.vector.tensor_copy(touch_v, bq_t[:, 0:1])
        nc.vector.tensor_copy(touch_v, bk_t[:, 0:1])
        nc.vector.tensor_copy(touch_v, mm_t[:, 0:1])
        touch_a = consts.tile([128, 1], F32)
        nc.scalar.copy(touch_a, mb_t[:, 0:1])

        ident16 = consts.tile([128, 128], F16)
        make_identity(nc, ident16)
        ident32 = consts.tile([128, 128], F32)
        make_identity(nc, ident32)
        ones1 = consts.tile([1, 128], F16)
        nc.vector.memset(ones1, 1.0)
        eps_t = consts.tile([128, 1], F32)
        nc.vector.memset(eps_t, 1e-5)

        # ---- persistent activations -----------------------------------
        # x and xres are loaded once into persistent tiles (streamed pool
        # slots would give the reload DMAs >2 sync waits, which the
        # direct2D DMA pseudo-instruction cannot encode).
        x_all = persist.tile([128, NC, D], F32)
        xv = x_d[:].rearrange("(o p) d -> p o d", p=128)
        for q in range(4):
            nc.sync.dma_start(x_all[:, 4 * q:4 * (q + 1), :],
                              xv[:, 4 * q:4 * (q + 1), :])
        xres_all = persist.tile([128, NC, CC], F32)
        nc.sync.dma_start(xres_all,
                          xres_d[:].rearrange("(o p) c -> p o c", p=128))
        xnT = persist.tile([128, DC, T], F16)       # xn transposed [d, n]
        qT = persist.tile([128, 2, T], F16)         # Q^T per head-pair
        kT = persist.tile([128, 2, T], F16)
        vP = persist.tile([128, NC, HC * (DH + 1)], BF16)  # V' with ones cols

        # ones columns of V' (softmax denominator trick)
        vP4 = vP[:].rearrange("p i (h c) -> p i h c", c=DH + 1)
        nc.vector.memset(vP4[:, :, :, DH], 1.0)

        # ---- phase 1: LayerNorm + transpose ---------------------------
        for ic in range(NC):
            x_t = x_all[:, ic, :]
            st = stats.tile([128, 2, 6], F32, tag="st")
            nc.vector.bn_stats(st[:, 0, :], x_t[:, 0:512])
            nc.vector.bn_stats(st[:, 1, :], x_t[:, 512:1024])
            mv = stats.tile([128, 2], F32, tag="mv")
            nc.vector.bn_aggr(mv, st)
            rstd = stats.tile([128, 1], F32, tag="rstd")
            nc.scalar.activation(rstd, mv[:, 1:2],
                                 mybir.ActivationFunctionType.Sqrt,
                                 bias=eps_t, scale=1.0)
            nc.vector.reciprocal(rstd, rstd)
            xc = xcpool.tile([128, D], F16, tag="xc")
            nc.vector.tensor_scalar(
                out=xc, in0=x_t, scalar1=mv[:, 0:1], scalar2=rstd,
                op0=mybir.AluOpType.subtract, op1=mybir.AluOpType.mult)
            for dc in range(DC):
                ps = tppsum.tile([128, 128], F16, tag="tp")
                nc.tensor.transpose(ps, xc[:, 128 * dc:128 * (dc + 1)], ident16)
                nc.vector.tensor_scalar(
                    out=xnT[:, dc, 128 * ic:128 * (ic + 1)], in0=ps,
                    scalar1=lngb_t[:, dc, 0:1], scalar2=lngb_t[:, dc, 1:2],
                    op0=mybir.AluOpType.mult, op1=mybir.AluOpType.add)

        # ---- phase 2a: V projection (+bias, *mask, bf16) ---------------
        for ic in range(NC):
            psv = mmpsum.tile([128, 512], F32, tag="mm")
            for dc in range(DC):
                nc.tensor.matmul(psv[:, :CC],
                                 lhsT=xnT[:, dc, 128 * ic:128 * (ic + 1)],
                                 rhs=wv_sb[:, dc, :],
                                 start=(dc == 0), stop=False)
            # rank-1 bias add: ones[1,128].T @ bv[1,CC]
            nc.tensor.matmul(psv[:, :CC], lhsT=ones1, rhs=bvr_t,
                             start=False, stop=True)
            for h in range(HC):
                nc.vector.tensor_scalar_mul(
                    out=vP[:, ic, (DH + 1) * h:(DH + 1) * h + DH],
                    in0=psv[:, DH * h:DH * (h + 1)],
                    scalar1=mm_t[:, ic:ic + 1])

        # ---- phase 2b + 3: per head-pair QK projection + attention -----
        for pg in range(2):
            for j in range(NJ):
                psk = mmpsum.tile([128, 512], F32, tag="mm")
                for dc in range(DC):
                    nc.tensor.matmul(psk,
                                     lhsT=wk_sb[:, dc, 128 * pg:128 * (pg + 1)],
                                     rhs=xnT[:, dc, 512 * j:512 * (j + 1)],
                                     start=(dc == 0), stop=(dc == DC - 1))
                nc.vector.tensor_scalar_add(
                    out=kT[:, pg, 512 * j:512 * (j + 1)], in0=psk,
                    scalar1=bk_t[:, pg:pg + 1])
                psq = mmpsum.tile([128, 512], F32, tag="mm")
                for dc in range(DC):
                    nc.tensor.matmul(psq,
                                     lhsT=wq_sb[:, dc, 128 * pg:128 * (pg + 1)],
                                     rhs=xnT[:, dc, 512 * j:512 * (j + 1)],
                                     start=(dc == 0), stop=(dc == DC - 1))
                nc.vector.tensor_scalar_add(
                    out=qT[:, pg, 512 * j:512 * (j + 1)], in0=psq,
                    scalar1=bq_t[:, pg:pg + 1])

            for j in range(NJ):
                nsl = slice(512 * j, 512 * (j + 1))
                ypsA = avpsum.tile([DH + 1, 512], F32, tag="av")
                ypsB = avpsum.tile([DH + 1, 512], F32, tag="av")
                for ic in range(NC):
                    msl = slice(128 * ic, 128 * (ic + 1))
                    sA = mmpsum.tile([128, 512], F32, tag="mm")
                    nc.tensor.matmul(sA, lhsT=kT[0:DH, pg, msl],
                                     rhs=qT[0:DH, pg, nsl],
                                     start=True, stop=True)
                    pA = ppool.tile([128, 512], BF16, tag="p")
                    nc.scalar.activation(pA, sA,
                                         mybir.ActivationFunctionType.Exp,
                                         bias=mb_t[:, ic:ic + 1], scale=1.0)
                    sB = mmpsum.tile([128, 512], F32, tag="mm")
                    nc.tensor.matmul(sB, lhsT=kT[DH:128, pg, msl],
                                     rhs=qT[DH:128, pg, nsl],
                                     start=True, stop=True)
                    pB = ppool.tile([128, 512], BF16, tag="p")
                    nc.scalar.activation(pB, sB,
                                         mybir.ActivationFunctionType.Exp,
                                         bias=mb_t[:, ic:ic + 1], scale=1.0)
                    hA = 2 * pg
                    hB = 2 * pg + 1
                    nc.tensor.matmul(
                        ypsA, lhsT=vP[:, ic, (DH + 1) * hA:(DH + 1) * (hA + 1)],
                        rhs=pA, start=(ic == 0), stop=(ic == NC - 1))
                    nc.tensor.matmul(
                        ypsB, lhsT=vP[:, ic, (DH + 1) * hB:(DH + 1) * (hB + 1)],
                        rhs=pB, start=(ic == 0), stop=(ic == NC - 1))

                # normalize + residual + store
                ytA = ytpool.tile([DH + 1, 512], F32, tag="yt")
                nc.vector.tensor_copy(ytA, ypsA)
                ytB = ytpool.tile([DH + 1, 512], F32, tag="yt")
                nc.vector.tensor_copy(ytB, ypsB)
                for k in range(4):
                    ic_g = 4 * j + k
                    rows = slice(128 * ic_g, 128 * (ic_g + 1))
                    ksl = slice(128 * k, 128 * (k + 1))
                    out_t = outpool.tile([128, 128], F32, tag="out")
                    for hh, yt in ((0, ytA), (1, ytB)):
                        otp = tppsum.tile([128, DH + 1], F32, tag="tp")
                        nc.tensor.transpose(otp, yt[:, ksl],
                                            ident32[0:DH + 1, 0:DH + 1])
                        rec = recpool.tile([128, 1], F32, tag="rec")
                        nc.vector.reciprocal(rec, otp[:, DH:DH + 1])
                        nc.vector.tensor_scalar_mul(
                            out=out_t[:, DH * hh:DH * (hh + 1)],
                            in0=otp[:, 0:DH], scalar1=rec)
                    nc.vector.tensor_add(
                        out_t, out_t,
                        xres_all[:, ic_g, 128 * pg:128 * (pg + 1)])
                    nc.sync.dma_start(
                        out_d[rows, 128 * pg:128 * (pg + 1)], out_t)


def _host_in_map(core, x, src_mask, ln_g, ln_b, Wq, bq, Wk, bk, Wv, bv):
    b, hg = divmod(core, 4)
    cs = CC * hg
    xb = np.ascontiguousarray(x[b], dtype=np.float32)
    mask = np.asarray(src_mask[b, :, 0], dtype=np.float32)

    def wslice(W):
        return np.ascontiguousarray(
            np.asarray(W, np.float32)[cs:cs + CC, :].T).astype(np.float16)

    return {
        "x": xb,
        "xres": np.ascontiguousarray(xb[:, cs:cs + CC]),
        "wqt": wslice(Wq),
        "wkt": wslice(Wk),
        "wvt": wslice(Wv),
        "lngb": np.ascontiguousarray(np.stack(
            [np.asarray(ln_g, np.float32).reshape(DC, 128).T,
             np.asarray(ln_b, np.float32).reshape(DC, 128).T], axis=-1)),
        "bq2": np.ascontiguousarray(
            np.asarray(bq, np.float32)[cs:cs + CC].reshape(2, 128).T),
        "bk2": np.ascontiguousarray(
            np.asarray(bk, np.float32)[cs:cs + CC].reshape(2, 128).T),
        "bvr": np.asarray(bv, np.float32)[cs:cs + CC].reshape(1, CC)
        .astype(np.float16),
        "mbias": np.ascontiguousarray(
            ((1.0 - mask) * -1000000.0).reshape(NC, 128).T),
        "mmul": np.ascontiguousarray(mask.reshape(NC, 128).T),
    }


def kernel(x, src_mask, ln_g, ln_b, Wq, bq, Wk, bk, Wv, bv, _trace=False,
           _tmpdir=None):
    x = np.asarray(x, dtype=np.float32)
    B = x.shape[0]
    if "nc" not in _CACHE:
        _CACHE["nc"] = build_bass()
    nc = _CACHE["nc"]

    from concourse.bass_utils import run_bass_kernel_spmd
    in_maps = [
        _host_in_map(c, x, np.asarray(src_mask), np.asarray(ln_g),
                     np.asarray(ln_b), np.asarray(Wq), np.asarray(bq),
                     np.asarray(Wk), np.asarray(bk), np.asarray(Wv),
                     np.asarray(bv))
        for c in range(8)
    ]
    res = run_bass_kernel_spmd(nc, in_maps, core_ids=list(range(8)),
                               trace=_trace, tmpdir=_tmpdir)
    out = np.empty((B, T, D), dtype=np.float32)
    for c in range(8):
        b, hg = divmod(c, 4)
        out[b, :, CC * hg:CC * (hg + 1)] = res.results[c]["out"]
    if _trace:
        _CACHE["last_result"] = res
    return out
